# revision 58
# baseline (speedup 1.0000x reference)
"""HKLinear (moe_routing) Trainium2 kernel — 8-core SPMD, data-parallel over tokens.

Math (reference):
    x = input.reshape(n, in_f)                       n=8192, in_f=4096
    sm = softmax((x @ centroids.T) / T)              [n, 64], T=0.1
    hits = sm > 0.01
    query_sel = any(hits, axis=1)   -> provably ALL TRUE (max softmax >= 1/64 > 0.01)
    cluster_sel = any(hits, axis=0)                  [64]  (global over ALL tokens)
    row_sel = cluster_sel[assignments]               [out_f]
    out = (x @ W.T + b) * (query_sel & row_sel)      [n, out_f]

Strategy: shard tokens 8 ways (1024/core), replicate W (bf16 everywhere; fp8
DoubleRow was measured at 2x/MAC but >=2 fp8 terms are needed to meet the
error budget, which ties bf16 -- no win). Per core the PE stream is kept dense
end-to-end at the (power-throttled) matmul roofline:
  - HAM warm-up dummies bridge the initial DMA wait;
  - ct + x stream in 8 k-plane chunk pairs on the sync HWDGE ring; routing
    matmuls (both token halves col-packed via tile_position) and main n=0
    (w0 prefetched early) interleave per chunk, so the PE never idles on DMA;
  - per-cluster hit flags come from exp -> PE ones-matmul token-sum ->
    threshold; a one-hot matmul (stacked twice, contracting both halves)
    gathers per-ROW hit counts; the [128, 32] counts are AllReduce-max'd
    across cores, consumed ONLY by the DVE epilogue -- the PE never waits
    on the collective;
  - epilogue: Scalar drains psum to SBUF (mask-free, fast psum recycle),
    DVE applies mask+bias, outputs store via gpsimd SWDGE while the
    collective may be pending, then the scalar HWDGE ring; the last tile
    masks psum directly and stores on the idle sync ring for a short tail.
Host does layout transposes + bf16 casts (free; HW exec time is what counts).
"""

import numpy as np
import ml_dtypes

N_CORES = 8
IN_F = 4096
OUT_F = 4096
N_CLUSTERS = 64
THRESHOLD = 0.01
TEMPERATURE = 0.1
N_TOKENS = 8192               # 4 * 2048
TOK_PER_CORE = N_TOKENS // N_CORES  # 1024

KT = IN_F // 128              # 32 k-tiles
NT = OUT_F // 128             # 32 out-feature tiles (psum partition dim)
MT = TOK_PER_CORE // 512      # 2 token tiles of 512 (moving free dim)
EXP_SHIFT = -30.0             # softmax-invariant shift, keeps exp() small
XCHUNKS = 8                   # x DMA split into k-plane chunks for early routing
KC = KT // XCHUNKS            # k-tiles per chunk

BF16 = ml_dtypes.bfloat16


def _build_bass():
    import concourse.bass as bass
    import concourse.mybir as mybir
    import concourse.tile as tile
    from concourse import bacc
    from concourse.bass import ds

    f32 = mybir.dt.float32
    bf16 = mybir.dt.bfloat16

    nc = bacc.Bacc("TRN2", target_bir_lowering=False, debug=False,
                   num_devices=N_CORES)

    # ---- DRAM I/O (per-core shards / replicated operands) ----
    xk_d = nc.dram_tensor("xk", [128, KT, TOK_PER_CORE], bf16, kind="ExternalInput")
    # w for n=0,1 separately (early prefetch); the rest as 2-tile pairs so
    # the per-tile dependency check (which breaks LDW pull-ahead for one MM
    # slot) happens every 128 MMs instead of 64
    wt01_d = nc.dram_tensor("wt01", [2, 128, KT, 128], bf16, kind="ExternalInput")
    wtp_d = nc.dram_tensor("wtp", [NT // 2 - 1, 128, KT, 2, 128], bf16,
                           kind="ExternalInput")
    ct_d = nc.dram_tensor("ct", [128, KT, N_CLUSTERS], bf16, kind="ExternalInput")
    ac_d = nc.dram_tensor("ac", [128, NT, 128], bf16, kind="ExternalInput")
    bc_d = nc.dram_tensor("bc", [128, NT], f32, kind="ExternalInput")
    out_d = nc.dram_tensor("out", [NT, MT, 128, 512], f32, kind="ExternalOutput")

    with tile.TileContext(nc) as tc:
        with (
            tc.tile_pool(name="resident", bufs=1) as resident,
            tc.tile_pool(name="wpool", bufs=5) as wpool,
            tc.tile_pool(name="opool", bufs=6) as opool,
            tc.tile_pool(name="route_sb", bufs=1) as route_sb,
            tc.tile_pool(name="psum_main", bufs=6, space="PSUM") as psum_main,
            tc.tile_pool(name="psum_route", bufs=1, space="PSUM") as psum_route,
            tc.tile_pool(name="cc_dram", bufs=1, space="DRAM") as cc_dram,
        ):
            # ---- input stream split over both HWDGE rings, priority order:
            # ct (routing stationary) -> x in k-chunks -> weights ----
            ct_sb = resident.tile([128, KT, N_CLUSTERS], bf16)
            x_sb = resident.tile([128, KT, TOK_PER_CORE], bf16)
            w0_sb = wpool.tile([128, KT, 128], bf16, tag="w_sb0", bufs=1,
                               name="w_sb_0")
            w1_sb = wpool.tile([128, KT, 128], bf16, tag="w_sb1", bufs=1,
                               name="w_sb_1")
            a_sb = resident.tile([128, NT, 128], bf16)
            bc_sb = resident.tile([128, NT], f32)
            for g in range(XCHUNKS):
                # ct k-slice just ahead of its x chunk: routing group g can
                # start as soon as these two small transfers land. w0/w1 and
                # the one-hot/bias tensors ride behind the early chunks,
                # where the PE (not DMA) is already the binder.
                nc.sync.dma_start(ct_sb[:, ds(g * KC, KC), :],
                                  ct_d[:, ds(g * KC, KC), :])
                nc.sync.dma_start(x_sb[:, ds(g * KC, KC), :],
                                  xk_d[:, ds(g * KC, KC), :])
                if g == 0:
                    nc.sync.dma_start(w0_sb[:], wt01_d[0, :, :, :])
                elif g == 2:
                    nc.sync.dma_start(w1_sb[:], wt01_d[1, :, :, :])
                elif g == 4:
                    nc.sync.dma_start(a_sb[:], ac_d[:])
                elif g == 6:
                    nc.sync.dma_start(bc_sb[:], bc_d[:])
            # small constants
            shift_col = route_sb.tile([128, 1], f32)
            nc.vector.memset(shift_col[:], EXP_SHIFT)
            ones_sb = route_sb.tile([128, N_CLUSTERS], bf16)
            nc.vector.memset(ones_sb[:], 1.0)

            # ---- HAM warm-up: dummy matmuls on a zeroed tile while the
            # first x chunk streams in; results are discarded ----
            warm_sb = route_sb.tile([128, 512], bf16)
            nc.vector.memset(warm_sb[:], 0.0)
            # reuses the psum_l bank: warm MMs complete (PE in-order) before
            # the routing accumulation claims the slot
            psum_w = psum_route.tile([128, 512], f32, tag="psum_l",
                                     name="psum_w")
            N_WARM = 12
            for i in range(N_WARM):
                nc.tensor.matmul(psum_w[:], warm_sb[:, 0:128], warm_sb[:],
                                 start=True, stop=True)

            # ---- routing: local per-cluster max margin, k-chunk interleaved.
            # Main-matmul n=0 k-groups trail one chunk behind the routing
            # groups, filling the PE holes while x chunks stream in. ----
            # both token halves col-packed into one [128, 512] psum:
            # partitions 0-63 = clusters x tokens[0:512], 64-127 = [512:1024]
            psum_l = psum_route.tile([128, 512], f32, tag="psum_l",
                                     name="psum_l")
            psums_n0 = [psum_main.tile([128, 512], f32, tag="psum_d",
                                       name=f"psum_d_0_{m}")
                        for m in range(MT)]
            psums_n1 = [psum_main.tile([128, 512], f32, tag="psum_d",
                                       name=f"psum_d_1_{m}")
                        for m in range(MT)]

            def main_mms(w_t, psums_t, k_lo, k_hi):
                for k in range(k_lo, k_hi):
                    for m in range(MT):
                        nc.tensor.matmul(
                            psums_t[m][:],
                            w_t[:, k, :],
                            x_sb[:, k, ds(m * 512, 512)],
                            start=(k == 0), stop=(k == KT - 1),
                        )

            for g in range(XCHUNKS):
                for k in range(g * KC, (g + 1) * KC):
                    for mt in range(MT):
                        nc.tensor.matmul(
                            psum_l[mt * 64:(mt + 1) * 64, :],
                            ct_sb[:, k, :],                    # lhsT [128, 64]
                            x_sb[:, k, ds(mt * 512, 512)],     # rhs  [128, 512]
                            start=(k == 0), stop=(k == KT - 1),
                            tile_position=(0, mt * 64),
                        )
                main_mms(w0_sb, psums_n0, g * KC, (g + 1) * KC)
                if g >= 2:
                    # n=1 trails two chunks behind (fills DMA-pacing holes)
                    main_mms(w1_sb, psums_n1, (g - 2) * KC, (g - 1) * KC)
            main_mms(w1_sb, psums_n1, (XCHUNKS - 2) * KC, XCHUNKS * KC)
            # e = exp(l + EXP_SHIFT) in bf16; S = sum_c e via PE ones-matmuls,
            # one per 64-partition half, running concurrently in separate
            # row/col groups (avoids the slow gpsimd partition reduce)
            e_sb = route_sb.tile([128, 512], bf16)
            nc.scalar.activation(e_sb[:], psum_l[:],
                                 mybir.ActivationFunctionType.Exp,
                                 bias=shift_col[:], scale=1.0)
            s_ps = psum_route.tile([128, 512], f32, tag="psum_l",
                                   name="s_ps")
            for mt in range(MT):
                nc.tensor.matmul(s_ps[mt * 64:(mt + 1) * 64, :],
                                 ones_sb[mt * 64:(mt + 1) * 64, :],
                                 e_sb[mt * 64:(mt + 1) * 64, :],
                                 start=True, stop=True,
                                 tile_position=(mt * 64, mt * 64))
            # hits test: e > thr*S ; cm[c (+64)] = max_t (e - thr*S)
            d_sb = route_sb.tile([128, 512], f32)
            nc.vector.scalar_tensor_tensor(
                d_sb[:], s_ps[:], float(-THRESHOLD), e_sb[:],
                op0=mybir.AluOpType.mult, op1=mybir.AluOpType.add)
            cm = route_sb.tile([128, 1], f32)
            nc.vector.reduce_max(cm[:], d_sb[:], axis=mybir.AxisListType.X)
            # per-half cluster hit flags (1.0/0.0); partitions 0-63 = half 0,
            # 64-127 = half 1 of the same 64 clusters
            sel_f = route_sb.tile([128, 1], f32)
            nc.vector.tensor_scalar(sel_f[:], cm[:], 0.0, None,
                                    op0=mybir.AluOpType.is_gt)
            sel_bf = route_sb.tile([128, 1], bf16)
            nc.vector.tensor_copy(sel_bf[:], sel_f[:])

            mask_sb = route_sb.tile([128, NT], f32)
            bmask_sb = route_sb.tile([128, NT], f32)

            # ---- one-hot gather of LOCAL row hit counts (PE, tiny):
            # a_sb stacks the one-hot twice, so the matmul contracts both
            # halves: count[p, nn] = sel0[assign] + sel1[assign] ----
            psum_m = psum_route.tile([128, NT], f32, tag="psum_m")
            for nn in range(NT):
                nc.tensor.matmul(psum_m[:, ds(nn, 1)], a_sb[:, nn, :],
                                 sel_bf[:], start=True, stop=True)
            margin_loc = route_sb.tile([128, NT], f32)
            nc.scalar.activation(margin_loc[:], psum_m[:],
                                 mybir.ActivationFunctionType.Copy)

            # AllReduce(max) of [128, NT] row hit counts across 8 cores
            cc_in = cc_dram.tile([128, NT], f32)
            cc_out = cc_dram.tile([128, NT], f32, addr_space="Shared")
            nc.gpsimd.dma_start(cc_in[:], margin_loc[:])
            nc.gpsimd.collective_compute(
                "AllReduce", mybir.AluOpType.max,
                replica_groups=[list(range(N_CORES))],
                ins=[cc_in.opt()], outs=[cc_out.opt()],
            )
            margin_red = route_sb.tile([128, NT], f32)
            nc.gpsimd.dma_start(margin_red[:], cc_out[:])

            # row mask 1.0/0.0 and mask*bias (DVE-only consumers)
            nc.vector.tensor_scalar(mask_sb[:], margin_red[:], 0.5,
                                    None, op0=mybir.AluOpType.is_gt)
            nc.vector.tensor_tensor(bmask_sb[:], mask_sb[:], bc_sb[:],
                                    op=mybir.AluOpType.mult)

            # ---- main matmul: out[feat_tile, tok] = W.T @ x  (bf16).
            # Epilogue is decoupled from the collective: Scalar drains psum
            # into o_raw (mask-free), DVE applies mask+bias, gpsimd stores.
            # The one-hot margin gather + AllReduce are slotted after n=1 so
            # the PE never waits on them. ----
            # Output DMAs: gpsimd SWDGE while the mask/collective may still
            # be pending (its FIFO never feeds back into the psum path),
            # scalar HWDGE ring later (faster). The last tile masks psum
            # directly on DVE and stores via the idle sync ring (psum
            # recycling no longer matters, minimal tail).
            def epilogue(n, psums):
                for m in range(MT):
                    o_sb = opool.tile([128, 512], f32, tag="o_sb", bufs=6,
                                      name=f"o_sb_{n}_{m}")
                    if n == NT - 1:
                        # one DVE op per m; stores split across both idle
                        # HWDGE rings so the two issues run in parallel
                        nc.vector.tensor_scalar(
                            o_sb[:], psums[m][:],
                            mask_sb[:, ds(n, 1)], bmask_sb[:, ds(n, 1)],
                            op0=mybir.AluOpType.mult,
                            op1=mybir.AluOpType.add,
                        )
                        eng = nc.sync if m == 0 else nc.scalar
                        eng.dma_start(out_d[n, m, :, :], o_sb[:])
                        continue
                    o_raw = opool.tile([128, 512], f32, tag="o_raw", bufs=10,
                                       name=f"o_raw_{n}_{m}")
                    nc.scalar.activation(o_raw[:], psums[m][:],
                                         mybir.ActivationFunctionType.Copy)
                    # out = raw * mask[n] + bias*mask[n]
                    nc.vector.tensor_scalar(
                        o_sb[:], o_raw[:],
                        mask_sb[:, ds(n, 1)], bmask_sb[:, ds(n, 1)],
                        op0=mybir.AluOpType.mult, op1=mybir.AluOpType.add,
                    )
                    if n < 8:
                        nc.gpsimd.dma_start(out_d[n, m, :, :], o_sb[:])
                    else:
                        nc.scalar.dma_start(out_d[n, m, :, :], o_sb[:])

            epilogue(0, psums_n0)
            epilogue(1, psums_n1)
            for j2 in range(NT // 2 - 1):
                w_sb = wpool.tile([128, KT, 2, 128], bf16, tag="w_sb2",
                                  bufs=3, name=f"w_sb2_{j2}")
                nc.sync.dma_start(w_sb[:], wtp_d[j2, :, :, :, :])
                for v in range(2):
                    n = 2 + 2 * j2 + v
                    psums = [psum_main.tile([128, 512], f32, tag="psum_d",
                                            name=f"psum_d_{n}_{m}")
                             for m in range(MT)]
                    for k in range(KT):
                        for m in range(MT):
                            nc.tensor.matmul(
                                psums[m][:],
                                w_sb[:, k, v, :],             # lhsT [128,128]
                                x_sb[:, k, ds(m * 512, 512)],  # rhs [128,512]
                                start=(k == 0), stop=(k == KT - 1),
                            )
                    epilogue(n, psums)

    nc.compile()
    return nc


_NC_CACHE = None


def _get_nc():
    global _NC_CACHE
    if _NC_CACHE is None:
        _NC_CACHE = _build_bass()
    return _NC_CACHE


def _prep_in_maps(input, weight, bias, centroids, assignments):
    x = np.ascontiguousarray(np.asarray(input, dtype=np.float32).reshape(N_TOKENS, IN_F))
    w = np.asarray(weight, dtype=np.float32)
    b = np.asarray(bias, dtype=np.float32)
    c = np.asarray(centroids, dtype=np.float32)
    a = np.asarray(assignments)

    # wt[n, p, k, j] = W.T[k*128+p, n*128+j] = W[n*128+j, k*128+p]
    wt = np.ascontiguousarray(
        w.T.reshape(KT, 128, NT, 128).transpose(2, 1, 0, 3)
    ).astype(BF16)
    # n=0,1 separate; n>=2 as pairs: wtp[j2, p, k, v, j] = wt[2+2*j2+v, p, k, j]
    wt01 = np.ascontiguousarray(wt[:2])
    wtp = np.ascontiguousarray(
        wt[2:].reshape(NT // 2 - 1, 2, 128, KT, 128).transpose(0, 2, 3, 1, 4))
    # ct[p, k, c] = centroids[c, k*128+p] / T
    ct = np.ascontiguousarray(
        (c / TEMPERATURE).T.reshape(KT, 128, N_CLUSTERS).transpose(1, 0, 2)
    ).astype(BF16)
    # one-hot stacked twice (contract both 64-partition halves of the
    # per-half hit flags): ac[c, n, j] = (assignments[n*128+j] == c % 64)
    ac1 = (a[None, :] == np.arange(N_CLUSTERS, dtype=a.dtype)[:, None])
    ac = np.concatenate([ac1, ac1], axis=0)
    ac = np.ascontiguousarray(ac.reshape(128, NT, 128)).astype(BF16)
    # bias columns: bc[p, n] = bias[n*128+p]
    bc = np.ascontiguousarray(b.reshape(NT, 128).T).astype(np.float32)

    in_maps = []
    for core in range(N_CORES):
        xs = x[core * TOK_PER_CORE:(core + 1) * TOK_PER_CORE]  # [1024, 4096]
        # xk[p, k, t] = x_shard[t, k*128+p]
        xk = np.ascontiguousarray(
            xs.T.reshape(KT, 128, TOK_PER_CORE).transpose(1, 0, 2)
        ).astype(BF16)
        in_maps.append({"xk": xk, "wt01": wt01, "wtp": wtp, "ct": ct,
                        "ac": ac, "bc": bc})
    return in_maps


def _assemble(results):
    # per-core out: [NT, MT, 128, 512] -> [1024 tokens, 4096 features]
    parts = []
    for core in range(N_CORES):
        oc = results[core]["out"]  # [32, 2, 128, 512]
        parts.append(oc.transpose(1, 3, 0, 2).reshape(TOK_PER_CORE, OUT_F))
    out = np.concatenate(parts, axis=0)  # [8192, 4096]
    return out.reshape(4, 2048, OUT_F).astype(np.float32)


def kernel(input, weight, bias, centroids, assignments):
    from concourse.bass_utils import run_bass_kernel_spmd

    nc = _get_nc()
    in_maps = _prep_in_maps(input, weight, bias, centroids, assignments)
    res = run_bass_kernel_spmd(nc, in_maps, core_ids=list(range(N_CORES)))
    return _assemble(res.results)


# revision 59
# speedup vs baseline: 1.0404x; 1.0404x over previous
"""HKLinear (moe_routing) Trainium2 kernel — 8-core SPMD, data-parallel over tokens.

Math (reference):
    x = input.reshape(n, in_f)                       n=8192, in_f=4096
    sm = softmax((x @ centroids.T) / T)              [n, 64], T=0.1
    hits = sm > 0.01
    query_sel = any(hits, axis=1)   -> provably ALL TRUE (max softmax >= 1/64 > 0.01)
    cluster_sel = any(hits, axis=0)                  [64]  (global over ALL tokens)
    row_sel = cluster_sel[assignments]               [out_f]
    out = (x @ W.T + b) * (query_sel & row_sel)      [n, out_f]

Strategy: shard tokens 8 ways (1024/core), replicate W (bf16 everywhere; fp8
DoubleRow was measured at 2x/MAC but >=2 fp8 terms are needed to meet the
error budget, which ties bf16 -- no win). Per core the PE stream is kept dense
end-to-end at the (power-throttled) matmul roofline:
  - HAM warm-up dummies bridge the initial DMA wait;
  - ct + x stream in 8 k-plane chunk pairs on the sync HWDGE ring; routing
    matmuls (both token halves col-packed via tile_position) and main n=0
    (w0 prefetched early) interleave per chunk, so the PE never idles on DMA;
  - per-cluster hit flags come from exp -> PE ones-matmul token-sum ->
    threshold; a one-hot matmul (stacked twice, contracting both halves)
    gathers per-ROW hit counts; the [128, 32] counts are AllReduce-max'd
    across cores, consumed ONLY by the DVE epilogue -- the PE never waits
    on the collective;
  - epilogue: Scalar drains psum to SBUF (mask-free, fast psum recycle),
    DVE applies mask+bias, outputs store via gpsimd SWDGE while the
    collective may be pending, then the scalar HWDGE ring; the last tile
    masks psum directly and stores on the idle sync ring for a short tail.
Host does layout transposes + bf16 casts (free; HW exec time is what counts).
"""

import numpy as np
import ml_dtypes

N_CORES = 8
IN_F = 4096
OUT_F = 4096
N_CLUSTERS = 64
THRESHOLD = 0.01
TEMPERATURE = 0.1
N_TOKENS = 8192               # 4 * 2048
TOK_PER_CORE = N_TOKENS // N_CORES  # 1024

KT = IN_F // 128              # 32 k-tiles
NT = OUT_F // 128             # 32 out-feature tiles (psum partition dim)
MT = TOK_PER_CORE // 512      # 2 token tiles of 512 (moving free dim)
EXP_SHIFT = -30.0             # softmax-invariant shift, keeps exp() small
XCHUNKS = 8                   # x DMA split into k-plane chunks for early routing
KC = KT // XCHUNKS            # k-tiles per chunk

BF16 = ml_dtypes.bfloat16


def _build_bass():
    import concourse.bass as bass
    import concourse.mybir as mybir
    import concourse.tile as tile
    from concourse import bacc
    from concourse.bass import ds

    f32 = mybir.dt.float32
    bf16 = mybir.dt.bfloat16

    nc = bacc.Bacc("TRN2", target_bir_lowering=False, debug=False,
                   num_devices=N_CORES)

    # ---- DRAM I/O (per-core shards / replicated operands) ----
    xk_d = nc.dram_tensor("xk", [128, KT, TOK_PER_CORE], bf16, kind="ExternalInput")
    # w for n=0,1 separately (early prefetch); the rest as 2-tile pairs so
    # the per-tile dependency check (which breaks LDW pull-ahead for one MM
    # slot) happens every 128 MMs instead of 64
    wt01_d = nc.dram_tensor("wt01", [2, 128, KT, 128], bf16, kind="ExternalInput")
    wtp_d = nc.dram_tensor("wtp", [NT // 2 - 1, 128, KT, 2, 128], bf16,
                           kind="ExternalInput")
    ct_d = nc.dram_tensor("ct", [128, KT, N_CLUSTERS], bf16, kind="ExternalInput")
    ac_d = nc.dram_tensor("ac", [128, NT, 128], bf16, kind="ExternalInput")
    bc_d = nc.dram_tensor("bc", [128, NT], f32, kind="ExternalInput")
    out_d = nc.dram_tensor("out", [NT, MT, 128, 512], f32, kind="ExternalOutput")

    with tile.TileContext(nc) as tc:
        with (
            tc.tile_pool(name="resident", bufs=1) as resident,
            tc.tile_pool(name="wpool", bufs=5) as wpool,
            tc.tile_pool(name="opool", bufs=6) as opool,
            tc.tile_pool(name="route_sb", bufs=1) as route_sb,
            tc.tile_pool(name="psum_main", bufs=6, space="PSUM") as psum_main,
            tc.tile_pool(name="psum_route", bufs=1, space="PSUM") as psum_route,
            tc.tile_pool(name="cc_dram", bufs=1, space="DRAM") as cc_dram,
        ):
            # ---- input stream split over both HWDGE rings, priority order:
            # ct (routing stationary) -> x in k-chunks -> weights ----
            ct_sb = resident.tile([128, KT, N_CLUSTERS], bf16)
            x_sb = resident.tile([128, KT, TOK_PER_CORE], bf16)
            w0_sb = wpool.tile([128, KT, 128], bf16, tag="w_sb0", bufs=1,
                               name="w_sb_0")
            w1_sb = wpool.tile([128, KT, 128], bf16, tag="w_sb1", bufs=1,
                               name="w_sb_1")
            a_sb = resident.tile([128, NT, 128], bf16)
            bc_sb = resident.tile([128, NT], f32)
            for g in range(XCHUNKS):
                # ct k-slice just ahead of its x chunk: routing group g can
                # start as soon as these two small transfers land. w0/w1 and
                # the one-hot/bias tensors ride behind the early chunks,
                # where the PE (not DMA) is already the binder.
                nc.sync.dma_start(ct_sb[:, ds(g * KC, KC), :],
                                  ct_d[:, ds(g * KC, KC), :])
                nc.sync.dma_start(x_sb[:, ds(g * KC, KC), :],
                                  xk_d[:, ds(g * KC, KC), :])
                if g == 0:
                    nc.sync.dma_start(w0_sb[:], wt01_d[0, :, :, :])
                elif g == 2:
                    nc.sync.dma_start(w1_sb[:], wt01_d[1, :, :, :])
                elif g == 4:
                    nc.sync.dma_start(a_sb[:], ac_d[:])
                elif g == 6:
                    nc.sync.dma_start(bc_sb[:], bc_d[:])
            # small constants
            shift_col = route_sb.tile([128, 1], f32)
            nc.vector.memset(shift_col[:], EXP_SHIFT)
            ones_sb = route_sb.tile([128, N_CLUSTERS], bf16)
            nc.vector.memset(ones_sb[:], 1.0)

            # ---- HAM warm-up: dummy matmuls on a zeroed tile while the
            # first x chunk streams in; results are discarded ----
            warm_sb = route_sb.tile([128, 512], bf16)
            nc.vector.memset(warm_sb[:], 0.0)
            # reuses the psum_l bank: warm MMs complete (PE in-order) before
            # the routing accumulation claims the slot
            psum_w = psum_route.tile([128, 512], f32, tag="psum_l",
                                     name="psum_w")
            N_WARM = 12
            for i in range(N_WARM):
                nc.tensor.matmul(psum_w[:], warm_sb[:, 0:128], warm_sb[:],
                                 start=True, stop=True)

            # ---- routing: local per-cluster max margin, k-chunk interleaved.
            # Main-matmul n=0 k-groups trail one chunk behind the routing
            # groups, filling the PE holes while x chunks stream in. ----
            # both token halves col-packed into one [128, 512] psum:
            # partitions 0-63 = clusters x tokens[0:512], 64-127 = [512:1024]
            psum_l = psum_route.tile([128, 512], f32, tag="psum_l",
                                     name="psum_l")
            psums_n0 = [psum_main.tile([128, 512], f32, tag="psum_d",
                                       name=f"psum_d_0_{m}")
                        for m in range(MT)]
            psums_n1 = [psum_main.tile([128, 512], f32, tag="psum_d",
                                       name=f"psum_d_1_{m}")
                        for m in range(MT)]

            def main_mms(w_t, psums_t, k_lo, k_hi):
                for k in range(k_lo, k_hi):
                    for m in range(MT):
                        nc.tensor.matmul(
                            psums_t[m][:],
                            w_t[:, k, :],
                            x_sb[:, k, ds(m * 512, 512)],
                            start=(k == 0), stop=(k == KT - 1),
                        )

            for g in range(XCHUNKS):
                for k in range(g * KC, (g + 1) * KC):
                    for mt in range(MT):
                        nc.tensor.matmul(
                            psum_l[mt * 64:(mt + 1) * 64, :],
                            ct_sb[:, k, :],                    # lhsT [128, 64]
                            x_sb[:, k, ds(mt * 512, 512)],     # rhs  [128, 512]
                            start=(k == 0), stop=(k == KT - 1),
                            tile_position=(0, mt * 64),
                        )
                main_mms(w0_sb, psums_n0, g * KC, (g + 1) * KC)
                if g >= 2:
                    # n=1 trails two chunks behind (fills DMA-pacing holes)
                    main_mms(w1_sb, psums_n1, (g - 2) * KC, (g - 1) * KC)
            main_mms(w1_sb, psums_n1, (XCHUNKS - 2) * KC, XCHUNKS * KC)
            # e = exp(l + EXP_SHIFT) in bf16; S = sum_c e via PE ones-matmuls,
            # one per 64-partition half, running concurrently in separate
            # row/col groups (avoids the slow gpsimd partition reduce)
            e_sb = route_sb.tile([128, 512], bf16)
            nc.scalar.activation(e_sb[:], psum_l[:],
                                 mybir.ActivationFunctionType.Exp,
                                 bias=shift_col[:], scale=1.0)
            s_ps = psum_route.tile([128, 512], f32, tag="psum_l",
                                   name="s_ps")
            for mt in range(MT):
                nc.tensor.matmul(s_ps[mt * 64:(mt + 1) * 64, :],
                                 ones_sb[mt * 64:(mt + 1) * 64, :],
                                 e_sb[mt * 64:(mt + 1) * 64, :],
                                 start=True, stop=True,
                                 tile_position=(mt * 64, mt * 64))
            # hits test: e > thr*S ; cm[c (+64)] = max_t (e - thr*S)
            d_sb = route_sb.tile([128, 512], f32)
            nc.vector.scalar_tensor_tensor(
                d_sb[:], s_ps[:], float(-THRESHOLD), e_sb[:],
                op0=mybir.AluOpType.mult, op1=mybir.AluOpType.add)
            cm = route_sb.tile([128, 1], f32)
            nc.vector.reduce_max(cm[:], d_sb[:], axis=mybir.AxisListType.X)
            # per-half cluster hit flags (1.0/0.0); partitions 0-63 = half 0,
            # 64-127 = half 1 of the same 64 clusters
            sel_f = route_sb.tile([128, 1], f32)
            nc.vector.tensor_scalar(sel_f[:], cm[:], 0.0, None,
                                    op0=mybir.AluOpType.is_gt)
            sel_bf = route_sb.tile([128, 1], bf16)
            nc.vector.tensor_copy(sel_bf[:], sel_f[:])

            mask_sb = route_sb.tile([128, NT], f32)
            bmask_sb = route_sb.tile([128, NT], f32)

            # ---- one-hot gather of LOCAL row hit counts (PE, tiny):
            # a_sb stacks the one-hot twice, so the matmul contracts both
            # halves: count[p, nn] = sel0[assign] + sel1[assign] ----
            psum_m = psum_route.tile([128, NT], f32, tag="psum_m")
            for nn in range(NT):
                nc.tensor.matmul(psum_m[:, ds(nn, 1)], a_sb[:, nn, :],
                                 sel_bf[:], start=True, stop=True)
            margin_loc = route_sb.tile([128, NT], f32)
            nc.scalar.activation(margin_loc[:], psum_m[:],
                                 mybir.ActivationFunctionType.Copy)

            # AllReduce(max) of [128, NT] row hit counts across 8 cores
            cc_in = cc_dram.tile([128, NT], f32)
            cc_out = cc_dram.tile([128, NT], f32, addr_space="Shared")
            nc.gpsimd.dma_start(cc_in[:], margin_loc[:])
            nc.gpsimd.collective_compute(
                "AllReduce", mybir.AluOpType.max,
                replica_groups=[list(range(N_CORES))],
                ins=[cc_in.opt()], outs=[cc_out.opt()],
            )
            margin_red = route_sb.tile([128, NT], f32)
            nc.gpsimd.dma_start(margin_red[:], cc_out[:])

            # row mask 1.0/0.0 and mask*bias (DVE-only consumers)
            nc.vector.tensor_scalar(mask_sb[:], margin_red[:], 0.5,
                                    None, op0=mybir.AluOpType.is_gt)
            nc.vector.tensor_tensor(bmask_sb[:], mask_sb[:], bc_sb[:],
                                    op=mybir.AluOpType.mult)

            # ---- main matmul: out[feat_tile, tok] = W.T @ x  (bf16).
            # Epilogue is decoupled from the collective: Scalar drains psum
            # into o_raw (mask-free), DVE applies mask+bias, gpsimd stores.
            # The one-hot margin gather + AllReduce are slotted after n=1 so
            # the PE never waits on them. ----
            # Output DMAs: gpsimd SWDGE while the mask/collective may still
            # be pending (its FIFO never feeds back into the psum path),
            # scalar HWDGE ring later (faster). The last tile masks psum
            # directly on DVE and stores via the idle sync ring (psum
            # recycling no longer matters, minimal tail).
            def epilogue(n, psums):
                for m in range(MT):
                    o_sb = opool.tile([128, 512], f32, tag="o_sb", bufs=6,
                                      name=f"o_sb_{n}_{m}")
                    if n == NT - 1:
                        # half-column chunks pipeline DVE against DMA issue
                        for c in range(2):
                            nc.vector.tensor_scalar(
                                o_sb[:, ds(c * 256, 256)],
                                psums[m][:, ds(c * 256, 256)],
                                mask_sb[:, ds(n, 1)], bmask_sb[:, ds(n, 1)],
                                op0=mybir.AluOpType.mult,
                                op1=mybir.AluOpType.add,
                            )
                            nc.sync.dma_start(out_d[n, m, :, ds(c * 256, 256)],
                                              o_sb[:, ds(c * 256, 256)])
                        continue
                    o_raw = opool.tile([128, 512], f32, tag="o_raw", bufs=10,
                                       name=f"o_raw_{n}_{m}")
                    nc.scalar.activation(o_raw[:], psums[m][:],
                                         mybir.ActivationFunctionType.Copy)
                    # out = raw * mask[n] + bias*mask[n]
                    nc.vector.tensor_scalar(
                        o_sb[:], o_raw[:],
                        mask_sb[:, ds(n, 1)], bmask_sb[:, ds(n, 1)],
                        op0=mybir.AluOpType.mult, op1=mybir.AluOpType.add,
                    )
                    if n < 8:
                        nc.gpsimd.dma_start(out_d[n, m, :, :], o_sb[:])
                    else:
                        nc.scalar.dma_start(out_d[n, m, :, :], o_sb[:])

            epilogue(0, psums_n0)
            epilogue(1, psums_n1)
            for j2 in range(NT // 2 - 1):
                w_sb = wpool.tile([128, KT, 2, 128], bf16, tag="w_sb2",
                                  bufs=3, name=f"w_sb2_{j2}")
                nc.sync.dma_start(w_sb[:], wtp_d[j2, :, :, :, :])
                for v in range(2):
                    n = 2 + 2 * j2 + v
                    psums = [psum_main.tile([128, 512], f32, tag="psum_d",
                                            name=f"psum_d_{n}_{m}")
                             for m in range(MT)]
                    for k in range(KT):
                        for m in range(MT):
                            nc.tensor.matmul(
                                psums[m][:],
                                w_sb[:, k, v, :],             # lhsT [128,128]
                                x_sb[:, k, ds(m * 512, 512)],  # rhs [128,512]
                                start=(k == 0), stop=(k == KT - 1),
                            )
                    epilogue(n, psums)

    nc.compile()
    return nc


_NC_CACHE = None


def _get_nc():
    global _NC_CACHE
    if _NC_CACHE is None:
        _NC_CACHE = _build_bass()
    return _NC_CACHE


def _prep_in_maps(input, weight, bias, centroids, assignments):
    x = np.ascontiguousarray(np.asarray(input, dtype=np.float32).reshape(N_TOKENS, IN_F))
    w = np.asarray(weight, dtype=np.float32)
    b = np.asarray(bias, dtype=np.float32)
    c = np.asarray(centroids, dtype=np.float32)
    a = np.asarray(assignments)

    # wt[n, p, k, j] = W.T[k*128+p, n*128+j] = W[n*128+j, k*128+p]
    wt = np.ascontiguousarray(
        w.T.reshape(KT, 128, NT, 128).transpose(2, 1, 0, 3)
    ).astype(BF16)
    # n=0,1 separate; n>=2 as pairs: wtp[j2, p, k, v, j] = wt[2+2*j2+v, p, k, j]
    wt01 = np.ascontiguousarray(wt[:2])
    wtp = np.ascontiguousarray(
        wt[2:].reshape(NT // 2 - 1, 2, 128, KT, 128).transpose(0, 2, 3, 1, 4))
    # ct[p, k, c] = centroids[c, k*128+p] / T
    ct = np.ascontiguousarray(
        (c / TEMPERATURE).T.reshape(KT, 128, N_CLUSTERS).transpose(1, 0, 2)
    ).astype(BF16)
    # one-hot stacked twice (contract both 64-partition halves of the
    # per-half hit flags): ac[c, n, j] = (assignments[n*128+j] == c % 64)
    ac1 = (a[None, :] == np.arange(N_CLUSTERS, dtype=a.dtype)[:, None])
    ac = np.concatenate([ac1, ac1], axis=0)
    ac = np.ascontiguousarray(ac.reshape(128, NT, 128)).astype(BF16)
    # bias columns: bc[p, n] = bias[n*128+p]
    bc = np.ascontiguousarray(b.reshape(NT, 128).T).astype(np.float32)

    in_maps = []
    for core in range(N_CORES):
        xs = x[core * TOK_PER_CORE:(core + 1) * TOK_PER_CORE]  # [1024, 4096]
        # xk[p, k, t] = x_shard[t, k*128+p]
        xk = np.ascontiguousarray(
            xs.T.reshape(KT, 128, TOK_PER_CORE).transpose(1, 0, 2)
        ).astype(BF16)
        in_maps.append({"xk": xk, "wt01": wt01, "wtp": wtp, "ct": ct,
                        "ac": ac, "bc": bc})
    return in_maps


def _assemble(results):
    # per-core out: [NT, MT, 128, 512] -> [1024 tokens, 4096 features]
    parts = []
    for core in range(N_CORES):
        oc = results[core]["out"]  # [32, 2, 128, 512]
        parts.append(oc.transpose(1, 3, 0, 2).reshape(TOK_PER_CORE, OUT_F))
    out = np.concatenate(parts, axis=0)  # [8192, 4096]
    return out.reshape(4, 2048, OUT_F).astype(np.float32)


def kernel(input, weight, bias, centroids, assignments):
    from concourse.bass_utils import run_bass_kernel_spmd

    nc = _get_nc()
    in_maps = _prep_in_maps(input, weight, bias, centroids, assignments)
    res = run_bass_kernel_spmd(nc, in_maps, core_ids=list(range(N_CORES)))
    return _assemble(res.results)
